# revision 3
# baseline (speedup 1.0000x reference)
"""Trainium2 Bass kernel for nn_MultiHeadClassifier.

  logits[b, c] = sum_{(g,l): label_ids[g,l]==c} group_probs[b,g] *
                 (features[b] @ W[g,l] + b[g,l])

Data-parallel over batch (8 cores, 4096 rows each). Per core:
  * Host prep: transpose features/group_probs; sort the G*L=1024 head
    outputs by target class, pad so no class straddles a 128-row chunk
    -> NCH chunks with disjoint class bands covering [0, C).
  * GEMM1 (PE, bf16): glT[gl, b] = Wsorted^T.T @ X^T per chunk/b-tile.
  * M-matmul (PE, bf16): MT[gl, b] = E_j.T @ pT (group-prob gather as a
    0/1 matmul).
  * ACT: per-partition bias add + PSUM drain; DVE: weighted = gb * MT.
  * Scatter (PE, bf16): logits[b, lo_j:hi_j] = weightedT_j.T @ S_j with
    S_j a 0/1 band matrix; bands disjoint -> independent start=True
    matmuls, accumulation happens inside the band via duplicate class
    columns of S.
"""
import os
import sys
import numpy as np
import ml_dtypes

for _p in ("/opt/trn_rl_repo",):
    if _p not in sys.path:
        sys.path.append(_p)

import concourse.bass as bass  # noqa: E402
import concourse.tile as tile  # noqa: E402
from concourse import bacc, mybir, bass_utils  # noqa: E402
from contextlib import ExitStack  # noqa: E402

F32 = mybir.dt.float32
BF16 = mybir.dt.bfloat16

B, F, G, L, C = 32768, 512, 16, 64, 1000
NCORE = 8
BC = B // NCORE          # 4096 batch rows per core
NT = BC // 512           # 8 b-tiles of 512
KF = F // 128            # 4 feature chunks

LAST_EXEC_NS = None


def _ensure_ntff_hook():
    """Some images ship an `antenv` without the optional `axon_hooks`
    submodule; bass_utils then crashes on import when tracing. Provide
    the module and register the ctypes NTFF hook trn_boot would have."""
    try:
        from antenv import axon_hooks  # noqa: F401
        return
    except ImportError:
        pass
    import types
    import antenv
    mod = types.ModuleType("antenv.axon_hooks")
    _hook = [None]
    mod.set_axon_ntff_profile_hook = lambda h: _hook.__setitem__(0, h)
    mod.get_axon_ntff_profile_hook = lambda: _hook[0]
    sys.modules["antenv.axon_hooks"] = mod
    antenv.axon_hooks = mod
    try:
        from trn_agent_boot.trn_boot import _ntff_profile_via_ctypes
        h = _ntff_profile_via_ctypes("/opt/axon/libaxon_pjrt.so")
        if h is not None:
            mod.set_axon_ntff_profile_hook(h)
    except Exception:
        pass


def _host_prep(W, b, label_ids):
    lab = np.asarray(label_ids).reshape(-1).astype(np.int64)
    GL = lab.shape[0]
    order = np.argsort(lab, kind="stable")
    rows, cur = [], 0
    classes, starts = np.unique(lab[order], return_index=True)
    starts = list(starts) + [GL]
    for ci in range(len(classes)):
        seg = order[starts[ci]:starts[ci + 1]]
        if cur + len(seg) > 128:
            rows += [-1] * (128 - cur)
            cur = 0
        rows += list(seg)
        cur = (cur + len(seg)) % 128
    if len(rows) % 128:
        rows += [-1] * (128 - len(rows) % 128)
    rows = np.array(rows, dtype=np.int64)
    K_pad = len(rows)
    NCH = K_pad // 128

    his = []
    for j in range(NCH):
        rj = rows[j * 128:(j + 1) * 128]
        valid = rj[rj >= 0]
        his.append(int(lab[valid].max()) + 1 if len(valid) else (his[-1] if his else 0))
    his[-1] = C
    for j in range(1, NCH):
        his[j] = max(his[j], his[j - 1])
    los = [0] + his[:-1]
    bands = list(zip(los, his))

    S_cat = np.zeros((128, C), dtype=ml_dtypes.bfloat16)
    for j, (lo, hi) in enumerate(bands):
        rj = rows[j * 128:(j + 1) * 128]
        for r in range(128):
            gl = rj[r]
            if gl >= 0:
                S_cat[r, lab[gl]] = 1.0

    Wflat = np.asarray(W).reshape(GL, F)
    bflat = np.asarray(b).reshape(GL)
    WT = np.zeros((F, K_pad), dtype=np.float32)
    biasT = np.zeros((128, NCH), dtype=np.float32)
    E = np.zeros((16, K_pad), dtype=ml_dtypes.bfloat16)
    for p, gl in enumerate(rows):
        if gl >= 0:
            WT[:, p] = Wflat[gl]
            biasT[p % 128, p // 128] = bflat[gl]
            E[gl // L, p] = 1.0
    return dict(K_pad=K_pad, NCH=NCH, bands=bands, S_cat=S_cat,
                WT=WT.astype(ml_dtypes.bfloat16), biasT=biasT, E=E)


def _band_segments(lo, hi):
    """Split [lo, hi) at 512-column (PSUM bank) boundaries."""
    segs = []
    while lo < hi:
        nxt = min(hi, (lo // 512 + 1) * 512)
        segs.append((lo, nxt))
        lo = nxt
    return segs


def _build_program(NCH, bands):
    nc = bacc.Bacc("TRN2", target_bir_lowering=False, debug=False,
                   num_devices=NCORE)
    xt_d = nc.dram_tensor("xt", [F, BC], BF16, kind="ExternalInput").ap()
    pt_d = nc.dram_tensor("pt", [16, BC], BF16, kind="ExternalInput").ap()
    wt_d = nc.dram_tensor("wt", [F, NCH * 128], BF16, kind="ExternalInput").ap()
    e_d = nc.dram_tensor("e", [16, NCH * 128], BF16, kind="ExternalInput").ap()
    bt_d = nc.dram_tensor("bt", [128, NCH], F32, kind="ExternalInput").ap()
    s_d = nc.dram_tensor("s", [128, C], BF16, kind="ExternalInput").ap()
    out_d = nc.dram_tensor("logits", [BC, C], F32, kind="ExternalOutput").ap()

    with tile.TileContext(nc) as tc, ExitStack() as ctx:
        const = ctx.enter_context(tc.tile_pool(name="const", bufs=1))
        psG = ctx.enter_context(tc.tile_pool(name="psG", bufs=2, space="PSUM"))
        psM = ctx.enter_context(tc.tile_pool(name="psM", bufs=2, space="PSUM"))
        psL = ctx.enter_context(tc.tile_pool(name="psL", bufs=2, space="PSUM"))
        sbG = ctx.enter_context(tc.tile_pool(name="sbG", bufs=6))
        sbW = ctx.enter_context(tc.tile_pool(name="sbW", bufs=24))
        sbO = ctx.enter_context(tc.tile_pool(name="sbO", bufs=6))

        # X^T column-slices: separate tiles for precise DMA deps.
        xts = [[None] * NT for _ in range(KF)]

        def load_x(k, t):
            t_ = const.tile([128, 512], BF16, name=f"x{k}_{t}", tag=f"x{k}_{t}")
            nc.sync.dma_start(t_[:],
                              xt_d[k * 128:(k + 1) * 128, bass.ts(t, 512)])
            xts[k][t] = t_

        # interleave the tiles needed by the first GEMM (x slices of t=0 and
        # W chunks) so the PE can start as early as possible
        wts = []
        for k in range(KF):
            load_x(k, 0)
            t_ = const.tile([128, NCH * 128], BF16, name=f"wts{k}", tag=f"wts{k}")
            nc.gpsimd.dma_start(t_[:], wt_d[k * 128:(k + 1) * 128, :])
            wts.append(t_)
        pts = const.tile([16, BC], BF16, name="pts", tag="pts")
        nc.gpsimd.dma_start(pts[:], pt_d[:])
        es = const.tile([16, NCH * 128], BF16, name="es", tag="es")
        nc.gpsimd.dma_start(es[:], e_d[:])
        bts = const.tile([128, NCH], F32, name="bts", tag="bts")
        nc.gpsimd.dma_start(bts[:], bt_d[:])
        ss = const.tile([128, C], BF16, name="ss", tag="ss")
        nc.gpsimd.dma_start(ss[:], s_d[:])
        for t in range(1, NT):
            for k in range(KF):
                load_x(k, t)

        all_wtjs = {}

        def gemm_phase(t):
            bsl = bass.ts(t, 512)
            wtjs = []
            for j in range(NCH):
                jsl = bass.ts(j, 128)
                pg = psG.tile([128, 512], F32, name="pg", tag="pg")
                for k in range(KF):
                    nc.tensor.matmul(pg[:], wts[k][:, jsl], xts[k][t][:],
                                     start=(k == 0), stop=(k == KF - 1))
                pm = psM.tile([128, 512], F32, name="pm", tag="pm")
                nc.tensor.matmul(pm[:], es[:, jsl], pts[:, bsl],
                                 start=True, stop=True)
                gb = sbG.tile([128, 512], BF16, name="gb", tag="gb")
                nc.scalar.activation(gb[:], pg[:],
                                     mybir.ActivationFunctionType.Identity,
                                     bias=bts[:, j:j + 1], scale=1.0)
                wtj = sbW.tile([128, 512], BF16, name="wtj", tag="wtj")
                nc.vector.tensor_mul(wtj[:], gb[:], pm[:])
                wtjs.append(wtj)
            all_wtjs[t] = wtjs

        def scatter_phase(t):
            wtjs = all_wtjs.pop(t)
            for bs_i in range(4):
                pl = psL.tile([128, 1024], F32, name="pl", tag="pl")
                for j, (lo, hi) in enumerate(bands):
                    for (n0, n1) in _band_segments(lo, hi):
                        nc.tensor.matmul(pl[:, n0:n1],
                                         wtjs[j][:, bass.ts(bs_i, 128)],
                                         ss[:, n0:n1], start=True, stop=True)
                ob = sbO.tile([128, C], F32, name="ob", tag="ob")
                # split the PSUM drain per bank across both engines
                nc.scalar.activation(ob[:, :512], pl[:, :512],
                                     mybir.ActivationFunctionType.Identity,
                                     bias=0.0, scale=1.0)
                nc.vector.tensor_copy(ob[:, 512:C], pl[:, 512:C])
                # scalar-queue HWDGE: keep output stream off the input queue
                nc.scalar.dma_start(out_d[t * 512 + bs_i * 128:
                                          t * 512 + (bs_i + 1) * 128, :], ob[:])

        # software-pipelined emission: scatter(t-1) after gemm(t)
        for t in range(NT + 1):
            if t < NT:
                gemm_phase(t)
            if t > 0:
                scatter_phase(t - 1)
    nc.finalize()
    return nc


def kernel(features, group_probs, W, b, label_ids):
    global LAST_EXEC_NS
    features = np.asarray(features, dtype=np.float32)
    group_probs = np.asarray(group_probs, dtype=np.float32)
    prep = _host_prep(W, b, label_ids)
    nc = _build_program(prep["NCH"], prep["bands"])

    XT = np.ascontiguousarray(features.T.astype(ml_dtypes.bfloat16))
    PT = np.ascontiguousarray(group_probs.T.astype(ml_dtypes.bfloat16))
    in_maps = []
    for c in range(NCORE):
        in_maps.append({
            "xt": np.ascontiguousarray(XT[:, c * BC:(c + 1) * BC]),
            "pt": np.ascontiguousarray(PT[:, c * BC:(c + 1) * BC]),
            "wt": prep["WT"],
            "e": prep["E"],
            "bt": prep["biasT"],
            "s": prep["S_cat"],
        })

    trace = bool(os.environ.get("BASS_TRACE"))
    if trace:
        bass_utils.upload_artifacts = lambda d: "local://skipped"
        _ensure_ntff_hook()
    try:
        res = bass_utils.run_bass_kernel_spmd(nc, in_maps,
                                              core_ids=list(range(NCORE)))
    except Exception:
        # transient NRT device errors have been observed; one retry
        res = bass_utils.run_bass_kernel_spmd(nc, in_maps,
                                              core_ids=list(range(NCORE)))
    if trace:
        LAST_EXEC_NS = res.exec_time_ns
        if res.exec_time_ns is not None:
            print(f"HW exec time: {res.exec_time_ns} ns")

    out = np.concatenate([res.results[c]["logits"] for c in range(NCORE)],
                         axis=0)
    return np.ascontiguousarray(out.astype(np.float32))



# revision 9
# speedup vs baseline: 1.1359x; 1.1359x over previous
"""Trainium2 Bass kernel for nn_MultiHeadClassifier.

  logits[b, c] = sum_{(g,l): label_ids[g,l]==c} group_probs[b,g] *
                 (features[b] @ W[g,l] + b[g,l])

Data-parallel over batch (8 cores, 4096 rows each). Per core:
  * Host prep: sort the G*L=1024 head outputs by target class (no
    padding -> exactly NCH=8 chunks of 128). Classes straddling a chunk
    boundary are handled by 1-wide accumulate fixup matmuls.
  * GEMM1 (PE, bf16): pg[gl, b] = Wsorted^T.T @ X^T per chunk/b-tile.
  * M-matmul (PE, bf16): pm[gl, b] = E_j.T @ pT (group-prob gather as a
    0/1 matmul).
  * DVE scalar_tensor_tensor: wtj = (pg + bias) * pm, PSUM-direct.
  * Scatter (PE, bf16): pl[b, lo_j:hi_j] = wtjT_j.T @ S_j with S_j a
    0/1 band matrix; disjoint bands + fixup columns for straddlers.
  * Drain pl (PSUM f32) to SBUF split across ACT/DVE/Pool, DMA out.
"""
import os
import sys
import numpy as np
import ml_dtypes

for _p in ("/opt/trn_rl_repo",):
    if _p not in sys.path:
        sys.path.append(_p)

import concourse.bass as bass  # noqa: E402
import concourse.tile as tile  # noqa: E402
from concourse import bacc, mybir, bass_utils  # noqa: E402
from contextlib import ExitStack  # noqa: E402

F32 = mybir.dt.float32
BF16 = mybir.dt.bfloat16

B, F, G, L, C = 32768, 512, 16, 64, 1000
NCORE = 8
BC = B // NCORE          # 4096 batch rows per core
NT = BC // 512           # 8 b-tiles of 512
KF = F // 128            # 4 feature chunks
GL = G * L               # 1024 heads
NCH = GL // 128          # 8 head chunks, no padding

LAST_EXEC_NS = None


def _ensure_ntff_hook():
    """Some images ship an `antenv` without the optional `axon_hooks`
    submodule; bass_utils then crashes on import when tracing. Provide
    the module and register the ctypes NTFF hook trn_boot would have."""
    try:
        from antenv import axon_hooks  # noqa: F401
        return
    except ImportError:
        pass
    import types
    import antenv
    mod = types.ModuleType("antenv.axon_hooks")
    _hook = [None]
    mod.set_axon_ntff_profile_hook = lambda h: _hook.__setitem__(0, h)
    mod.get_axon_ntff_profile_hook = lambda: _hook[0]
    sys.modules["antenv.axon_hooks"] = mod
    antenv.axon_hooks = mod
    try:
        from trn_agent_boot.trn_boot import _ntff_profile_via_ctypes
        h = _ntff_profile_via_ctypes("/opt/axon/libaxon_pjrt.so")
        if h is not None:
            mod.set_axon_ntff_profile_hook(h)
    except Exception:
        pass


def _host_prep(W, b, label_ids):
    lab = np.asarray(label_ids).reshape(-1).astype(np.int64)
    order = np.argsort(lab, kind="stable")
    lab_s = lab[order]                      # ascending classes, len 1024

    # bands + straddle fixups
    bands = []
    fixups = []                             # (chunk_j, cstar, fix_col)
    lo = 0
    for j in range(NCH):
        if j < NCH - 1:
            c_end = int(lab_s[128 * (j + 1) - 1])
            c_next = int(lab_s[128 * (j + 1)])
            if c_end == c_next:
                hi = c_end + 1
                fixups.append((j + 1, c_end, len(fixups)))
            else:
                hi = c_next
        else:
            hi = C
        bands.append((lo, hi))
        lo = hi

    straddle_tail = {(j, c) for (j, c, _) in fixups}
    NFIX = max(len(fixups), 1)
    S_cat = np.zeros((128, C), dtype=ml_dtypes.bfloat16)
    S_fix = np.zeros((128, NFIX), dtype=ml_dtypes.bfloat16)
    fix_of = {(j, c): col for (j, c, col) in fixups}
    for p in range(GL):
        j, r = p // 128, p % 128
        c = int(lab_s[p])
        if (j, c) in straddle_tail:
            S_fix[r, fix_of[(j, c)]] = 1.0
        else:
            S_cat[r, c] = 1.0

    Wflat = np.asarray(W).reshape(GL, F).astype(np.float32)
    bflat = np.asarray(b).reshape(GL).astype(np.float32)
    Wsorted = Wflat[order]                  # [1024, F]
    # wt tiles: [KF, 2, 128, 512] -> contiguous [128,512] blocks
    WT2 = np.ascontiguousarray(
        Wsorted.T.reshape(KF, 128, GL).reshape(KF, 128, 2, 512)
        .transpose(0, 2, 1, 3)).astype(ml_dtypes.bfloat16)
    biasT = np.zeros((128, NCH), dtype=np.float32)
    E = np.zeros((16, GL), dtype=ml_dtypes.bfloat16)
    for p, gl in enumerate(order):
        biasT[p % 128, p // 128] = bflat[gl]
        E[gl // L, p] = 1.0
    return dict(bands=bands, fixups=fixups, NFIX=NFIX, S_cat=S_cat,
                S_fix=S_fix, WT2=WT2, biasT=biasT, E=E)


def _band_segments(lo, hi, fix_cols):
    """Split [lo, hi) at 512-col (PSUM bank) boundaries; a segment whose
    range contains a fixup target column keeps its accumulation open."""
    segs = []
    while lo < hi:
        nxt = min(hi, (lo // 512 + 1) * 512)
        stop = not any(lo <= c < nxt for c in fix_cols)
        segs.append((lo, nxt, stop))
        lo = nxt
    return segs


def _build_program(bands, fixups, NFIX):
    nc = bacc.Bacc("TRN2", target_bir_lowering=False, debug=False,
                   num_devices=NCORE)
    xt_d = nc.dram_tensor("xt", [KF * NT * 128, 512], BF16,
                          kind="ExternalInput").ap()
    pt_d = nc.dram_tensor("pt", [16, BC], BF16, kind="ExternalInput").ap()
    wt_d = nc.dram_tensor("wt", [KF * 2 * 128, 512], BF16,
                          kind="ExternalInput").ap()
    e_d = nc.dram_tensor("e", [16, GL], BF16, kind="ExternalInput").ap()
    bt_d = nc.dram_tensor("bt", [128, NCH], F32, kind="ExternalInput").ap()
    s_d = nc.dram_tensor("s", [128, C], BF16, kind="ExternalInput").ap()
    sf_d = nc.dram_tensor("sf", [128, NFIX], BF16, kind="ExternalInput").ap()
    out_d = nc.dram_tensor("logits", [BC, C], F32, kind="ExternalOutput").ap()

    # fixup targets per chunk: chunk j -> [(cstar, col)]
    fix_by_chunk = {}
    for (j, c, col) in fixups:
        fix_by_chunk.setdefault(j, []).append((c, col))
    # per chunk: columns that a LATER fixup will accumulate into
    fix_cols_of_band = {}
    for (j, c, col) in fixups:
        fix_cols_of_band.setdefault(j - 1, []).append(c)

    with tile.TileContext(nc) as tc, ExitStack() as ctx:
        const = ctx.enter_context(tc.tile_pool(name="const", bufs=1))
        psG = ctx.enter_context(tc.tile_pool(name="psG", bufs=2, space="PSUM"))
        psM = ctx.enter_context(tc.tile_pool(name="psM", bufs=2, space="PSUM"))
        psL = ctx.enter_context(tc.tile_pool(name="psL", bufs=2, space="PSUM"))
        sbW = ctx.enter_context(tc.tile_pool(name="sbW", bufs=18))
        sbP = ctx.enter_context(tc.tile_pool(name="sbP", bufs=4))
        sbO = ctx.enter_context(tc.tile_pool(name="sbO", bufs=6))

        # small consts on the gpsimd (software) queue; pts first (pm j=0 dep)
        pts = const.tile([16, BC], BF16, name="pts", tag="pts")
        nc.gpsimd.dma_start(pts[:], pt_d[:])
        es = const.tile([16, GL], BF16, name="es", tag="es")
        nc.gpsimd.dma_start(es[:], e_d[:])
        bts = const.tile([128, NCH], F32, name="bts", tag="bts")
        nc.gpsimd.dma_start(bts[:], bt_d[:])
        ss = const.tile([128, C], BF16, name="ss", tag="ss")
        nc.gpsimd.dma_start(ss[:], s_d[:])
        sfs = const.tile([128, NFIX], BF16, name="sfs", tag="sfs")
        nc.gpsimd.dma_start(sfs[:], sf_d[:])

        # weights + activations on the sync hw queue; first-gemm deps first
        wts = [[None, None] for _ in range(KF)]
        xts = [[None] * NT for _ in range(KF)]

        def load_wt(k, h):
            t_ = const.tile([128, 512], BF16, name=f"w{k}_{h}", tag=f"w{k}_{h}")
            nc.sync.dma_start(t_[:], wt_d[(k * 2 + h) * 128:
                                          (k * 2 + h + 1) * 128, :])
            wts[k][h] = t_

        def load_x(k, t):
            t_ = const.tile([128, 512], BF16, name=f"x{k}_{t}", tag=f"x{k}_{t}")
            nc.sync.dma_start(t_[:], xt_d[(k * NT + t) * 128:
                                          (k * NT + t + 1) * 128, :])
            xts[k][t] = t_

        for k in range(KF):
            load_wt(k, 0)
            load_x(k, 0)
        for k in range(KF):
            load_wt(k, 1)
        for t in range(1, NT):
            for k in range(KF):
                load_x(k, t)

        def wt_slice(k, j):
            """lhsT [128, 128] for feature-chunk k, head-chunk j."""
            h, off = divmod(j * 128, 512)
            return wts[k][h][:, off:off + 128]

        all_wtjs = {}

        def gemm_chunk(t, j):
            bsl = bass.ts(t, 512)
            pg = psG.tile([128, 512], F32, name="pg", tag="pg")
            for k in range(KF):
                nc.tensor.matmul(pg[:], wt_slice(k, j), xts[k][t][:],
                                 start=(k == 0), stop=(k == KF - 1))
            pm = psM.tile([128, 512], F32, name="pm", tag="pm")
            nc.tensor.matmul(pm[:], es[:, bass.ts(j, 128)], pts[:, bsl],
                             start=True, stop=True)
            # hw limit: only one PSUM operand per DVE op -> drain pg on ACT
            gb = sbP.tile([128, 512], BF16, name="gb", tag="gb")
            nc.scalar.activation(gb[:], pg[:],
                                 mybir.ActivationFunctionType.Identity,
                                 bias=bts[:, j:j + 1], scale=1.0)
            wtj = sbW.tile([128, 512], BF16, name="wtj", tag="wtj")
            nc.vector.tensor_mul(wtj[:], gb[:], pm[:])
            all_wtjs.setdefault(t, []).append(wtj)

        pending_drain = []

        def scatter_mm(t, bs_i):
            wtjs = all_wtjs[t]
            bsl = bass.ts(bs_i, 128)
            pl = psL.tile([128, 1024], F32, name="pl", tag="pl")
            for j, (lo, hi) in enumerate(bands):
                for (n0, n1, stop) in _band_segments(
                        lo, hi, fix_cols_of_band.get(j, ())):
                    nc.tensor.matmul(pl[:, n0:n1], wtjs[j][:, bsl],
                                     ss[:, n0:n1], start=True, stop=stop)
                # a later band's start=True re-marks the whole 2KB zero
                # region pending-zero, which would wipe the straddle column
                # -> the fixup accumulate must run before band j+1 starts
                for (cstar, col) in fix_by_chunk.get(j + 1, ()):
                    nc.tensor.matmul(pl[:, cstar:cstar + 1],
                                     wtjs[j + 1][:, bsl],
                                     sfs[:, col:col + 1],
                                     start=False, stop=True)
            pending_drain.append((t, bs_i, pl))

        def scatter_drain():
            while pending_drain:
                t, bs_i, pl = pending_drain.pop(0)
                ob = sbO.tile([128, C], F32, name="ob", tag="ob")
                # gpsimd cannot read PSUM; split the drain ACT/DVE
                nc.scalar.activation(ob[:, :512], pl[:, :512],
                                     mybir.ActivationFunctionType.Identity,
                                     bias=0.0, scale=1.0)
                nc.vector.tensor_copy(ob[:, 512:C], pl[:, 512:C])
                nc.sync.dma_start(out_d[t * 512 + bs_i * 128:
                                        t * 512 + (bs_i + 1) * 128, :], ob[:])

        # software pipeline: scatter(t-1) subtiles interleaved into gemm(t);
        # drains emitted one chunk later so gb/mult stay ahead in the
        # ACT/DVE queues
        for t in range(NT):
            for j in range(NCH):
                gemm_chunk(t, j)
                scatter_drain()
                if t > 0 and j % 2 == 1:
                    scatter_mm(t - 1, j // 2)
            if t > 0:
                all_wtjs.pop(t - 1)
        for bs_i in range(4):
            scatter_mm(NT - 1, bs_i)
            scatter_drain()
        all_wtjs.pop(NT - 1)
    nc.finalize()
    return nc


def kernel(features, group_probs, W, b, label_ids):
    global LAST_EXEC_NS
    features = np.asarray(features, dtype=np.float32)
    group_probs = np.asarray(group_probs, dtype=np.float32)
    prep = _host_prep(W, b, label_ids)
    nc = _build_program(prep["bands"], prep["fixups"], prep["NFIX"])

    Xb = features.astype(ml_dtypes.bfloat16)
    PT = np.ascontiguousarray(group_probs.T.astype(ml_dtypes.bfloat16))
    WT2 = np.ascontiguousarray(prep["WT2"].reshape(KF * 2 * 128, 512))
    in_maps = []
    for c in range(NCORE):
        Xc = Xb[c * BC:(c + 1) * BC]                      # [4096, 512]
        # [KF, NT, 128, 512]: tile (k, t) = Xc[t*512:(t+1)*512,
        #                                      k*128:(k+1)*128].T
        XT2 = np.ascontiguousarray(
            Xc.reshape(NT, 512, KF, 128).transpose(2, 0, 3, 1))
        in_maps.append({
            "xt": XT2.reshape(KF * NT * 128, 512),
            "pt": np.ascontiguousarray(PT[:, c * BC:(c + 1) * BC]),
            "wt": WT2,
            "e": prep["E"],
            "bt": prep["biasT"],
            "s": prep["S_cat"],
            "sf": prep["S_fix"],
        })

    trace = bool(os.environ.get("BASS_TRACE"))
    if trace:
        bass_utils.upload_artifacts = lambda d: "local://skipped"
        _ensure_ntff_hook()
    try:
        res = bass_utils.run_bass_kernel_spmd(nc, in_maps,
                                              core_ids=list(range(NCORE)))
    except Exception:
        # transient NRT device errors have been observed; one retry
        res = bass_utils.run_bass_kernel_spmd(nc, in_maps,
                                              core_ids=list(range(NCORE)))
    if trace:
        LAST_EXEC_NS = res.exec_time_ns
        if res.exec_time_ns is not None:
            print(f"HW exec time: {res.exec_time_ns} ns")

    out = np.concatenate([res.results[c]["logits"] for c in range(NCORE)],
                         axis=0)
    return np.ascontiguousarray(out.astype(np.float32))
